# revision 13
# baseline (speedup 1.0000x reference)
"""Causal attention kernel for trn2, sharded over 8 NeuronCores.

Problem (B=4, S=2048, E=2048, H=16, D=128), fp32 in/out:
    qkv = x @ w_qkv; q,k,v = split(qkv)
    q,k,v reshaped (B,S,E)->(B,H,S,D) as a RAW view (no transpose), i.e.
    per (b,h): Q_h = rows [h*128,(h+1)*128) of q[b] reinterpreted [S,D].
    o = softmax(QK^T/sqrt(D) + causal(+1/-10000)) @ V, inverse raw view,
    out = o @ w_out.

The raw view maps head h to a contiguous block of 128 sequence rows, so
the computation splits into B*H = 64 independent tasks; core c gets 8
tasks = rows [c*1024,(c+1)*1024) of x.reshape(B*S, E).  No collectives.

All matmuls run in bf16 (inputs converted and x pre-transposed host-
side); accumulation is fp32 in PSUM.  Attention computes S^T = K Q^T per
(q-chunk 512, k-tile 128) with causal tiles cut to their live q-range,
exp batched per [128,<=1024] PSUM generation, and the softmax
denominator built by gpsimd adds over the exp tiles + one f32r matmul
(ones stationary) per q-chunk to broadcast the partition-dim sum.
"""

import numpy as np

B, S, E = 4, 2048, 2048
H, D, P = 16, 128, 128
NCORES = 8
ROWS = B * S // NCORES   # 1024 rows per core = 8 tasks of 128 rows
NGRP = 2                 # task groups per core
NTT = 4                  # tasks per group
SCALE = float(1.0 / np.sqrt(D))
NEG = -1.0e9  # pre-scale additive mask; exp underflows to exactly 0.0

_NC_CACHE = {}


def build_nc():
    import concourse.mybir as mybir
    import concourse.tile as tile
    from concourse import bacc
    from concourse.masks import make_identity

    f32 = mybir.dt.float32
    f32r = mybir.dt.float32r
    bf16 = mybir.dt.bfloat16
    AF = mybir.ActivationFunctionType
    ALU = mybir.AluOpType

    nc = bacc.Bacc("TRN2", target_bir_lowering=False, debug=False,
                   num_devices=NCORES)
    # xt[kk, g, kc, ti*128+m] = x[row g*512+ti*128+m, kc*128+kk] (host-
    # pretransposed, bf16): DMAs straight into the matmul-ready layout.
    xt = nc.dram_tensor("xt", [P, NGRP * 16 * NTT * P], bf16,
                        kind="ExternalInput")
    wqkv = nc.dram_tensor("wqkv", [E, 3 * E], bf16, kind="ExternalInput")
    wout = nc.dram_tensor("wout", [E, E], bf16, kind="ExternalInput")
    out = nc.dram_tensor("out", [ROWS, E], f32, kind="ExternalOutput")

    xt_v = xt.ap().rearrange("p (g kc tm) -> p g kc tm", g=NGRP, kc=16)
    wqkv_v = wqkv.ap().rearrange("(ko p) c -> p ko c", p=P)   # [128,16,6144]
    wout_v = wout.ap().rearrange("(co p) n -> p co n", p=P)   # [128,16,2048]

    with tile.TileContext(nc) as tc:
        with (
            tc.tile_pool(name="const", bufs=1) as cpool,
            tc.tile_pool(name="atp", bufs=2) as atpool,
            tc.tile_pool(name="qk", bufs=1) as qkpool,
            tc.tile_pool(name="ot", bufs=4) as otpool,
            tc.tile_pool(name="wq", bufs=2) as wqpool,
            tc.tile_pool(name="wo", bufs=2) as wopool,
            tc.tile_pool(name="attw", bufs=3) as awpool,
            tc.tile_pool(name="vn", bufs=2) as vnpool,
            tc.tile_pool(name="osb", bufs=2) as ospool,
            tc.tile_pool(name="psQ", bufs=2, space="PSUM") as psQ,
            tc.tile_pool(name="ps2", bufs=2, space="PSUM") as ps2,
            tc.tile_pool(name="psO", bufs=2, space="PSUM") as psO,
        ):
            ident = cpool.tile([P, P], bf16, tag="ident")
            make_identity(nc, ident[:])
            # tri[kk, n] = 0 iff n >= kk else NEG: the within-tile causal
            # boundary (q-local offset n vs k-partition kk).
            tri = cpool.tile([P, P], f32, tag="tri")
            nc.gpsimd.memset(tri[:], 0.0)
            nc.gpsimd.affine_select(
                out=tri[:], in_=tri[:],
                compare_op=ALU.is_ge, fill=NEG,
                base=0, channel_multiplier=-1, pattern=[[1, P]],
            )
            # all-ones stationary (f32, matmul'd as f32r): den matmul
            # out[m,n] = sum_k ptsum[k,n] broadcast to all 128 partitions.
            ones = cpool.tile([P, P], bf16, tag="ones")
            nc.gpsimd.memset(ones[:], 1.0)

            at_g = [atpool.tile([P, 16, NTT * P], bf16, tag="at_all",
                                name=f"at{g}") for g in range(NGRP)]
            for kq in range(4):
                nc.sync.dma_start(at_g[0][:, kq * 4:(kq + 1) * 4, :],
                                  xt_v[:, 0, kq * 4:(kq + 1) * 4, :])

            for g in range(NGRP):
                qt_all = qkpool.tile([P, NTT, S], bf16, tag="qtc")
                kt_all = qkpool.tile([P, NTT, S], bf16, tag="ktc")
                vt_all = qkpool.tile([P, NTT, S], bf16, tag="vtc")
                dsts = (qt_all, kt_all, vt_all)

                # ---------------- QKV phase ----------------
                for cbp in range(24):
                    wq = wqpool.tile([P, 16, 2 * P], bf16, tag="wq")
                    nc.sync.dma_start(
                        wq[:], wqkv_v[:, :, cbp * 2 * P:(cbp + 1) * 2 * P])
                    for half in range(2):
                        cb = cbp * 2 + half
                        ps = psQ.tile([P, NTT * P], f32, tag="mm512")
                        for kc in range(16):
                            nc.tensor.matmul(
                                ps[:],
                                wq[:, kc, half * P:(half + 1) * P],
                                at_g[g][:, kc, :],
                                start=(kc == 0), stop=(kc == 15))
                        j = cb % 16
                        nc.vector.tensor_copy(
                            dsts[cb // 16].rearrange(
                                "d t (i j) -> d t i j", j=16)[:, :, :, j],
                            ps[:].rearrange("d (t m) -> d t m", t=NTT))

                # ---------------- attention phase (per task) ----------------
                if g + 1 < NGRP:
                    for kq in range(4):
                        nc.sync.dma_start(
                            at_g[g + 1][:, kq * 4:(kq + 1) * 4, :],
                            xt_v[:, g + 1, kq * 4:(kq + 1) * 4, :])
                wos = [wopool.tile([P, 16, 512], bf16, tag="wo",
                                   name=f"wo{nch}") for nch in range(4)]
                for nch in range(2):
                    nc.sync.dma_start(
                        wos[nch][:],
                        wout_v[:, :, nch * 512:(nch + 1) * 512])

                ots = []
                for ti in range(NTT):
                    # V natural tiles: vnat[kk, kt, d] = V[kt*128+kk, d]
                    vnat = vnpool.tile([P, 16, P], bf16, tag="vnat")
                    for half in range(2):
                        tp = psQ.tile([P, 8 * P], bf16, tag="mm512")
                        for sb in range(8):
                            kt = half * 8 + sb
                            nc.tensor.transpose(
                                tp[:, sb * P:(sb + 1) * P],
                                vt_all[:, ti, kt * P:(kt + 1) * P],
                                ident[:])
                        nc.vector.tensor_copy(
                            vnat[:, half * 8:(half + 1) * 8, :].rearrange(
                                "p s d -> p (s d)").bitcast(f32),
                            tp[:].bitcast(f32))

                    ot = otpool.tile([P, 16, P], bf16, tag="ot")  # O^T
                    ots.append(ot)
                    for qc in range(4):
                        # Generations: (kt, width, s2 col, q offset) entries
                        # packed into one [128,<=1024] PSUM tile + one exp.
                        # Full k-tiles in pairs; causal k-tiles kt=4qc+r only
                        # cover live q cols [r*128, 512).
                        gens = []
                        for gp in range(2 * qc):
                            gens.append([(2 * gp, 512, 0, 0),
                                         (2 * gp + 1, 512, 512, 0)])
                        gens.append([(4 * qc, 512, 0, 0),
                                     (4 * qc + 1, 384, 512, 128)])
                        gens.append([(4 * qc + 2, 256, 0, 256),
                                     (4 * qc + 3, 128, 256, 384)])

                        ot_ps = psQ.tile([P, 512], f32, tag="mm512")
                        den_ps = psO.tile([P, 512], f32, tag="denacc")

                        nge = len(gens)
                        for gi, entries in enumerate(gens):
                            diag = gi >= nge - 2
                            s2 = ps2.tile([P, 1024], f32, tag="s2")
                            for (kt, w, c0, q0) in entries:
                                nc.tensor.matmul(
                                    s2[:, c0:c0 + w],
                                    kt_all[:, ti, kt * P:(kt + 1) * P],
                                    qt_all[:, ti,
                                           qc * 512 + q0:qc * 512 + q0 + w],
                                    start=True, stop=True)
                            if diag:
                                for (kt, w, c0, q0) in entries:
                                    nc.vector.tensor_tensor(
                                        s2[:, c0:c0 + P], s2[:, c0:c0 + P],
                                        tri[:], ALU.add)
                            totw = entries[-1][2] + entries[-1][1]
                            pt = awpool.tile([P, 1024], bf16, tag="pt")
                            nc.scalar.activation(
                                pt[:, :totw], s2[:, :totw], AF.Exp,
                                bias=1.0, scale=SCALE)
                            for ei, (kt, w, c0, q0) in enumerate(entries):
                                first = gi == 0 and ei == 0
                                last = gi == nge - 1 and ei == 1
                                nc.tensor.matmul(
                                    ot_ps[:, q0:512],
                                    vnat[:, kt, :], pt[:, c0:c0 + w],
                                    start=first, stop=last,
                                    skip_group_check=True)
                                nc.tensor.matmul(
                                    den_ps[:, q0:512],
                                    ones[:], pt[:, c0:c0 + w],
                                    start=first, stop=last,
                                    skip_group_check=True)
                        rec = awpool.tile([P, 512], f32, tag="rec")
                        nc.vector.reciprocal_approx_fast(
                            out=rec[:], in_=den_ps[:])
                        nc.vector.tensor_tensor(
                            ot[:, qc * 4:(qc + 1) * 4, :].rearrange(
                                "p s d -> p (s d)"),
                            ot_ps[:], rec[:], ALU.mult)

                # ---------------- output projection ----------------
                for nch in range(2, 4):
                    nc.sync.dma_start(
                        wos[nch][:],
                        wout_v[:, :, nch * 512:(nch + 1) * 512])
                for nch in range(4):
                    for ti in range(NTT):
                        lt = ots[ti].rearrange("d qt (i j) -> d qt i j", j=16)
                        ps = psQ.tile([P, 512], f32, tag="mm512")
                        for cc in range(16):
                            nc.tensor.matmul(
                                ps[:], lt[:, :, :, cc],
                                wos[nch][:, cc, :],
                                start=(cc == 0), stop=(cc == 15))
                        osb = ospool.tile([P, 512], f32, tag="osb")
                        nc.vector.tensor_copy(osb[:], ps[:])
                        nc.scalar.dma_start(
                            out.ap()[(g * NTT + ti) * P:
                                     (g * NTT + ti + 1) * P,
                                     nch * 512:(nch + 1) * 512], osb[:])
    nc.compile()
    return nc


def get_nc():
    if "nc" not in _NC_CACHE:
        _NC_CACHE["nc"] = build_nc()
    return _NC_CACHE["nc"]


def make_in_maps(x, w_qkv, w_out):
    import ml_dtypes

    bf = ml_dtypes.bfloat16
    xf = np.ascontiguousarray(np.asarray(x, dtype=np.float32)).reshape(
        B * S, E).astype(bf)
    wqkv_b = np.ascontiguousarray(
        np.asarray(w_qkv, dtype=np.float32).astype(bf))
    wout_b = np.ascontiguousarray(
        np.asarray(w_out, dtype=np.float32).astype(bf))
    # xt[c][kk, g, kc, ti, m] = x[c*1024 + g*512 + ti*128 + m, kc*128 + kk]
    xa = xf.reshape(NCORES, NGRP, NTT, P, 16, P).transpose(0, 5, 1, 4, 2, 3)
    in_maps = [
        {"xt": np.ascontiguousarray(xa[c]).reshape(P, NGRP * 16 * NTT * P),
         "wqkv": wqkv_b, "wout": wout_b}
        for c in range(NCORES)
    ]
    return in_maps


def kernel(x, w_qkv, w_out):
    from concourse.bass_utils import run_bass_kernel_spmd

    nc = get_nc()
    in_maps = make_in_maps(x, w_qkv, w_out)
    res = run_bass_kernel_spmd(nc, in_maps, core_ids=list(range(NCORES)))
    outs = [res.results[c]["out"] for c in range(NCORES)]
    return np.concatenate(outs, axis=0).reshape(B, S, E).astype(np.float32)


# revision 14
# speedup vs baseline: 1.0651x; 1.0651x over previous
"""Causal attention kernel for trn2, sharded over 8 NeuronCores.

Problem (B=4, S=2048, E=2048, H=16, D=128), fp32 in/out:
    qkv = x @ w_qkv; q,k,v = split(qkv)
    q,k,v reshaped (B,S,E)->(B,H,S,D) as a RAW view (no transpose), i.e.
    per (b,h): Q_h = rows [h*128,(h+1)*128) of q[b] reinterpreted [S,D].
    o = softmax(QK^T/sqrt(D) + causal(+1/-10000)) @ V, inverse raw view,
    out = o @ w_out.

The raw view maps head h to a contiguous block of 128 sequence rows, so
the computation splits into B*H = 64 independent tasks; core c gets 8
tasks = rows [c*1024,(c+1)*1024) of x.reshape(B*S, E).  No collectives.

All matmuls run in bf16 (inputs converted and x pre-transposed host-
side); accumulation is fp32 in PSUM.  Attention computes S^T = K Q^T per
(q-chunk 512, k-tile 128) with causal tiles cut to their live q-range,
exp batched per [128,<=1024] PSUM generation, and the softmax
denominator built by gpsimd adds over the exp tiles + one f32r matmul
(ones stationary) per q-chunk to broadcast the partition-dim sum.
"""

import numpy as np

B, S, E = 4, 2048, 2048
H, D, P = 16, 128, 128
NCORES = 8
ROWS = B * S // NCORES   # 1024 rows per core = 8 tasks of 128 rows
NGRP = 2                 # task groups per core
NTT = 4                  # tasks per group
SCALE = float(1.0 / np.sqrt(D))
NEG = -1.0e9  # pre-scale additive mask; exp underflows to exactly 0.0

_NC_CACHE = {}


def build_nc():
    import concourse.mybir as mybir
    import concourse.tile as tile
    from concourse import bacc
    from concourse.masks import make_identity

    f32 = mybir.dt.float32
    f32r = mybir.dt.float32r
    bf16 = mybir.dt.bfloat16
    AF = mybir.ActivationFunctionType
    ALU = mybir.AluOpType

    nc = bacc.Bacc("TRN2", target_bir_lowering=False, debug=False,
                   num_devices=NCORES)
    # xt[kk, g, kc, ti*128+m] = x[row g*512+ti*128+m, kc*128+kk] (host-
    # pretransposed, bf16): DMAs straight into the matmul-ready layout.
    xt = nc.dram_tensor("xt", [P, NGRP * 16 * NTT * P], bf16,
                        kind="ExternalInput")
    wqkv = nc.dram_tensor("wqkv", [E, 3 * E], bf16, kind="ExternalInput")
    wout = nc.dram_tensor("wout", [E, E], bf16, kind="ExternalInput")
    out = nc.dram_tensor("out", [ROWS, E], f32, kind="ExternalOutput")

    xt_v = xt.ap().rearrange("p (g kc tm) -> p g kc tm", g=NGRP, kc=16)
    wqkv_v = wqkv.ap().rearrange("(ko p) c -> p ko c", p=P)   # [128,16,6144]
    wout_v = wout.ap().rearrange("(co p) n -> p co n", p=P)   # [128,16,2048]

    with tile.TileContext(nc) as tc:
        with (
            tc.tile_pool(name="const", bufs=1) as cpool,
            tc.tile_pool(name="atp", bufs=2) as atpool,
            tc.tile_pool(name="qk", bufs=1) as qkpool,
            tc.tile_pool(name="ot", bufs=4) as otpool,
            tc.tile_pool(name="wq", bufs=2) as wqpool,
            tc.tile_pool(name="wo", bufs=2) as wopool,
            tc.tile_pool(name="attw", bufs=5) as awpool,
            tc.tile_pool(name="recp", bufs=2) as rpool,
            tc.tile_pool(name="vn", bufs=2) as vnpool,
            tc.tile_pool(name="osb", bufs=2) as ospool,
            tc.tile_pool(name="psQ", bufs=2, space="PSUM") as psQ,
            tc.tile_pool(name="ps2", bufs=2, space="PSUM") as ps2,
            tc.tile_pool(name="psO", bufs=1, space="PSUM") as psO,
        ):
            ident = cpool.tile([P, P], bf16, tag="ident")
            make_identity(nc, ident[:])
            # tri[kk, n] = 0 iff n >= kk else NEG: the within-tile causal
            # boundary (q-local offset n vs k-partition kk).
            tri = cpool.tile([P, P], f32, tag="tri")
            nc.gpsimd.memset(tri[:], 0.0)
            nc.gpsimd.affine_select(
                out=tri[:], in_=tri[:],
                compare_op=ALU.is_ge, fill=NEG,
                base=0, channel_multiplier=-1, pattern=[[1, P]],
            )
            # all-ones stationary (f32, matmul'd as f32r): den matmul
            # out[m,n] = sum_k ptsum[k,n] broadcast to all 128 partitions.
            ones = cpool.tile([P, P], bf16, tag="ones")
            nc.gpsimd.memset(ones[:], 1.0)

            at_g = [atpool.tile([P, 16, NTT * P], bf16, tag="at_all",
                                name=f"at{g}") for g in range(NGRP)]
            for kq in range(4):
                nc.sync.dma_start(at_g[0][:, kq * 4:(kq + 1) * 4, :],
                                  xt_v[:, 0, kq * 4:(kq + 1) * 4, :])

            for g in range(NGRP):
                qt_all = qkpool.tile([P, NTT, S], bf16, tag="qtc")
                kt_all = qkpool.tile([P, NTT, S], bf16, tag="ktc")
                vt_all = qkpool.tile([P, NTT, S], bf16, tag="vtc")
                dsts = (qt_all, kt_all, vt_all)

                # ---------------- QKV phase ----------------
                for cbp in range(24):
                    wq = wqpool.tile([P, 16, 2 * P], bf16, tag="wq")
                    nc.sync.dma_start(
                        wq[:], wqkv_v[:, :, cbp * 2 * P:(cbp + 1) * 2 * P])
                    for half in range(2):
                        cb = cbp * 2 + half
                        ps = psQ.tile([P, NTT * P], f32, tag="mm512")
                        for kc in range(16):
                            nc.tensor.matmul(
                                ps[:],
                                wq[:, kc, half * P:(half + 1) * P],
                                at_g[g][:, kc, :],
                                start=(kc == 0), stop=(kc == 15))
                        j = cb % 16
                        nc.vector.tensor_copy(
                            dsts[cb // 16].rearrange(
                                "d t (i j) -> d t i j", j=16)[:, :, :, j],
                            ps[:].rearrange("d (t m) -> d t m", t=NTT))

                # ---------------- attention phase (per task) ----------------
                if g + 1 < NGRP:
                    for kq in range(4):
                        nc.sync.dma_start(
                            at_g[g + 1][:, kq * 4:(kq + 1) * 4, :],
                            xt_v[:, g + 1, kq * 4:(kq + 1) * 4, :])
                wos = [wopool.tile([P, 16, 512], bf16, tag="wo",
                                   name=f"wo{nch}") for nch in range(4)]
                for nch in range(2):
                    nc.sync.dma_start(
                        wos[nch][:],
                        wout_v[:, :, nch * 512:(nch + 1) * 512])

                ots = []
                for ti in range(NTT):
                    # V natural tiles: vnat[kk, kt, d] = V[kt*128+kk, d]
                    vnat = vnpool.tile([P, 16, P], bf16, tag="vnat")
                    for half in range(2):
                        tp = psQ.tile([P, 8 * P], bf16, tag="mm512")
                        for sb in range(8):
                            kt = half * 8 + sb
                            nc.tensor.transpose(
                                tp[:, sb * P:(sb + 1) * P],
                                vt_all[:, ti, kt * P:(kt + 1) * P],
                                ident[:])
                        nc.vector.tensor_copy(
                            vnat[:, half * 8:(half + 1) * 8, :].rearrange(
                                "p s d -> p (s d)").bitcast(f32),
                            tp[:].bitcast(f32))

                    ot = otpool.tile([P, 16, P], bf16, tag="ot")  # O^T
                    ots.append(ot)
                    for qc in range(4):
                        # Generations: (kt, width, s2 col, q offset) entries
                        # packed into one [128,<=1024] PSUM tile + one exp.
                        # Full k-tiles in pairs; causal k-tiles kt=4qc+r only
                        # cover live q cols [r*128, 512).
                        gens = []
                        for gp in range(2 * qc):
                            gens.append([(2 * gp, 512, 0, 0, False),
                                         (2 * gp + 1, 512, 512, 0, False)])
                        gens.append([(4 * qc, 512, 0, 0, True),
                                     (4 * qc + 1, 384, 512, 128, True)])
                        gens.append([(4 * qc + 2, 256, 0, 256, True),
                                     (4 * qc + 3, 128, 256, 384, True)])
                        nge = len(gens)

                        ot_ps = psO.tile([P, 512], f32, tag="otacc")
                        den_ps = psO.tile([P, 512], f32, tag="denacc")

                        pts = [None] * nge

                        def emit_qk(gi):
                            s2 = ps2.tile([P, 1024], f32, tag="s2")
                            totw = 0
                            for (kt, w, c0, q0, dg) in gens[gi]:
                                nc.tensor.matmul(
                                    s2[:, c0:c0 + w],
                                    kt_all[:, ti, kt * P:(kt + 1) * P],
                                    qt_all[:, ti,
                                           qc * 512 + q0:qc * 512 + q0 + w],
                                    start=True, stop=True)
                                totw = c0 + w
                            for (kt, w, c0, q0, dg) in gens[gi]:
                                if dg:
                                    nc.vector.tensor_tensor(
                                        s2[:, c0:c0 + P], s2[:, c0:c0 + P],
                                        tri[:], ALU.add)
                            pt = awpool.tile([P, 1024], bf16, tag="pt")
                            nc.scalar.activation(
                                pt[:, :totw], s2[:, :totw], AF.Exp,
                                bias=1.0, scale=SCALE)
                            pts[gi] = pt

                        def emit_pv(gi):
                            for ei, (kt, w, c0, q0, dg) in enumerate(gens[gi]):
                                first = gi == 0 and ei == 0
                                last = gi == nge - 1 and ei == 1
                                nc.tensor.matmul(
                                    ot_ps[:, q0:512],
                                    vnat[:, kt, :], pts[gi][:, c0:c0 + w],
                                    start=first, stop=last,
                                    skip_group_check=True)
                                nc.tensor.matmul(
                                    den_ps[:, q0:512],
                                    ones[:], pts[gi][:, c0:c0 + w],
                                    start=first, stop=last,
                                    skip_group_check=True)

                        emit_qk(0)
                        if nge > 1:
                            emit_qk(1)
                        for gi in range(nge):
                            emit_pv(gi)
                            if gi + 2 < nge:
                                emit_qk(gi + 2)
                        rec = rpool.tile([P, 512], f32, tag="rec")
                        nc.vector.reciprocal_approx_fast(
                            out=rec[:], in_=den_ps[:])
                        nc.vector.tensor_tensor(
                            ot[:, qc * 4:(qc + 1) * 4, :].rearrange(
                                "p s d -> p (s d)"),
                            ot_ps[:], rec[:], ALU.mult)

                # ---------------- output projection ----------------
                for nch in range(2, 4):
                    nc.sync.dma_start(
                        wos[nch][:],
                        wout_v[:, :, nch * 512:(nch + 1) * 512])
                for nch in range(4):
                    for ti in range(NTT):
                        lt = ots[ti].rearrange("d qt (i j) -> d qt i j", j=16)
                        ps = psQ.tile([P, 512], f32, tag="mm512")
                        for cc in range(16):
                            nc.tensor.matmul(
                                ps[:], lt[:, :, :, cc],
                                wos[nch][:, cc, :],
                                start=(cc == 0), stop=(cc == 15))
                        osb = ospool.tile([P, 512], f32, tag="osb")
                        nc.vector.tensor_copy(osb[:], ps[:])
                        nc.scalar.dma_start(
                            out.ap()[(g * NTT + ti) * P:
                                     (g * NTT + ti + 1) * P,
                                     nch * 512:(nch + 1) * 512], osb[:])
    nc.compile()
    return nc


def get_nc():
    if "nc" not in _NC_CACHE:
        _NC_CACHE["nc"] = build_nc()
    return _NC_CACHE["nc"]


def make_in_maps(x, w_qkv, w_out):
    import ml_dtypes

    bf = ml_dtypes.bfloat16
    xf = np.ascontiguousarray(np.asarray(x, dtype=np.float32)).reshape(
        B * S, E).astype(bf)
    wqkv_b = np.ascontiguousarray(
        np.asarray(w_qkv, dtype=np.float32).astype(bf))
    wout_b = np.ascontiguousarray(
        np.asarray(w_out, dtype=np.float32).astype(bf))
    # xt[c][kk, g, kc, ti, m] = x[c*1024 + g*512 + ti*128 + m, kc*128 + kk]
    xa = xf.reshape(NCORES, NGRP, NTT, P, 16, P).transpose(0, 5, 1, 4, 2, 3)
    in_maps = [
        {"xt": np.ascontiguousarray(xa[c]).reshape(P, NGRP * 16 * NTT * P),
         "wqkv": wqkv_b, "wout": wout_b}
        for c in range(NCORES)
    ]
    return in_maps


def kernel(x, w_qkv, w_out):
    from concourse.bass_utils import run_bass_kernel_spmd

    nc = get_nc()
    in_maps = make_in_maps(x, w_qkv, w_out)
    res = run_bass_kernel_spmd(nc, in_maps, core_ids=list(range(NCORES)))
    outs = [res.results[c]["out"] for c in range(NCORES)]
    return np.concatenate(outs, axis=0).reshape(B, S, E).astype(np.float32)


# revision 15
# speedup vs baseline: 1.0735x; 1.0078x over previous
"""Causal attention kernel for trn2, sharded over 8 NeuronCores.

Problem (B=4, S=2048, E=2048, H=16, D=128), fp32 in/out:
    qkv = x @ w_qkv; q,k,v = split(qkv)
    q,k,v reshaped (B,S,E)->(B,H,S,D) as a RAW view (no transpose), i.e.
    per (b,h): Q_h = rows [h*128,(h+1)*128) of q[b] reinterpreted [S,D].
    o = softmax(QK^T/sqrt(D) + causal(+1/-10000)) @ V, inverse raw view,
    out = o @ w_out.

The raw view maps head h to a contiguous block of 128 sequence rows, so
the computation splits into B*H = 64 independent tasks; core c gets 8
tasks = rows [c*1024,(c+1)*1024) of x.reshape(B*S, E).  No collectives.

All matmuls run in bf16 (inputs converted and x pre-transposed host-
side); accumulation is fp32 in PSUM.  Attention computes S^T = K Q^T per
(q-chunk 512, k-tile 128) with causal tiles cut to their live q-range,
exp batched per [128,<=1024] PSUM generation, and the softmax
denominator built by gpsimd adds over the exp tiles + one f32r matmul
(ones stationary) per q-chunk to broadcast the partition-dim sum.
"""

import numpy as np

B, S, E = 4, 2048, 2048
H, D, P = 16, 128, 128
NCORES = 8
ROWS = B * S // NCORES   # 1024 rows per core = 8 tasks of 128 rows
NGRP = 2                 # task groups per core
NTT = 4                  # tasks per group
SCALE = float(1.0 / np.sqrt(D))
NEG = -1.0e9  # pre-scale additive mask; exp underflows to exactly 0.0

_NC_CACHE = {}


def build_nc():
    import concourse.mybir as mybir
    import concourse.tile as tile
    from concourse import bacc
    from concourse.masks import make_identity

    f32 = mybir.dt.float32
    f32r = mybir.dt.float32r
    bf16 = mybir.dt.bfloat16
    AF = mybir.ActivationFunctionType
    ALU = mybir.AluOpType

    nc = bacc.Bacc("TRN2", target_bir_lowering=False, debug=False,
                   num_devices=NCORES)
    # xt[kk, g, kc, ti*128+m] = x[row g*512+ti*128+m, kc*128+kk] (host-
    # pretransposed, bf16): DMAs straight into the matmul-ready layout.
    xt = nc.dram_tensor("xt", [P, NGRP * 16 * NTT * P], bf16,
                        kind="ExternalInput")
    wqkv = nc.dram_tensor("wqkv", [E, 3 * E], bf16, kind="ExternalInput")
    wout = nc.dram_tensor("wout", [E, E], bf16, kind="ExternalInput")
    out = nc.dram_tensor("out", [ROWS, E], f32, kind="ExternalOutput")

    xt_v = xt.ap().rearrange("p (g kc tm) -> p g kc tm", g=NGRP, kc=16)
    wqkv_v = wqkv.ap().rearrange("(ko p) c -> p ko c", p=P)   # [128,16,6144]
    wout_v = wout.ap().rearrange("(co p) n -> p co n", p=P)   # [128,16,2048]

    with tile.TileContext(nc) as tc:
        with (
            tc.tile_pool(name="const", bufs=1) as cpool,
            tc.tile_pool(name="atp", bufs=2) as atpool,
            tc.tile_pool(name="qk", bufs=1) as qkpool,
            tc.tile_pool(name="ot", bufs=4) as otpool,
            tc.tile_pool(name="wq", bufs=2) as wqpool,
            tc.tile_pool(name="wo", bufs=2) as wopool,
            tc.tile_pool(name="attw", bufs=5) as awpool,
            tc.tile_pool(name="recp", bufs=2) as rpool,
            tc.tile_pool(name="vn", bufs=2) as vnpool,
            tc.tile_pool(name="osb", bufs=2) as ospool,
            tc.tile_pool(name="psQ", bufs=2, space="PSUM") as psQ,
            tc.tile_pool(name="ps2", bufs=2, space="PSUM") as ps2,
            tc.tile_pool(name="psO", bufs=1, space="PSUM") as psO,
        ):
            ident = cpool.tile([P, P], bf16, tag="ident")
            make_identity(nc, ident[:])
            # tri[kk, n] = 0 iff n >= kk else NEG: the within-tile causal
            # boundary (q-local offset n vs k-partition kk).
            tri = cpool.tile([P, P], f32, tag="tri")
            nc.gpsimd.memset(tri[:], 0.0)
            nc.gpsimd.affine_select(
                out=tri[:], in_=tri[:],
                compare_op=ALU.is_ge, fill=NEG,
                base=0, channel_multiplier=-1, pattern=[[1, P]],
            )
            # all-ones stationary (f32, matmul'd as f32r): den matmul
            # out[m,n] = sum_k ptsum[k,n] broadcast to all 128 partitions.
            ones = cpool.tile([P, P], bf16, tag="ones")
            nc.gpsimd.memset(ones[:], 1.0)

            at_g = [atpool.tile([P, 16, NTT * P], bf16, tag="at_all",
                                name=f"at{g}") for g in range(NGRP)]
            for kq in range(4):
                nc.sync.dma_start(at_g[0][:, kq * 4:(kq + 1) * 4, :],
                                  xt_v[:, 0, kq * 4:(kq + 1) * 4, :])

            for g in range(NGRP):
                qt_all = qkpool.tile([P, NTT, S], bf16, tag="qtc")
                kt_all = qkpool.tile([P, NTT, S], bf16, tag="ktc")
                vt_all = qkpool.tile([P, NTT, S], bf16, tag="vtc")
                dsts = (qt_all, kt_all, vt_all)

                # ---------------- QKV phase ----------------
                for cbp in range(24):
                    wq = wqpool.tile([P, 16, 2 * P], bf16, tag="wq")
                    nc.scalar.dma_start(
                        wq[:], wqkv_v[:, :, cbp * 2 * P:(cbp + 1) * 2 * P])
                    for half in range(2):
                        cb = cbp * 2 + half
                        ps = psQ.tile([P, NTT * P], f32, tag="mm512")
                        for kc in range(16):
                            nc.tensor.matmul(
                                ps[:],
                                wq[:, kc, half * P:(half + 1) * P],
                                at_g[g][:, kc, :],
                                start=(kc == 0), stop=(kc == 15))
                        j = cb % 16
                        nc.vector.tensor_copy(
                            dsts[cb // 16].rearrange(
                                "d t (i j) -> d t i j", j=16)[:, :, :, j],
                            ps[:].rearrange("d (t m) -> d t m", t=NTT))

                # ---------------- attention phase (per task) ----------------
                if g + 1 < NGRP:
                    for kq in range(4):
                        nc.sync.dma_start(
                            at_g[g + 1][:, kq * 4:(kq + 1) * 4, :],
                            xt_v[:, g + 1, kq * 4:(kq + 1) * 4, :])
                wos = [wopool.tile([P, 16, 512], bf16, tag="wo",
                                   name=f"wo{nch}") for nch in range(4)]
                for nch in range(2):
                    nc.sync.dma_start(
                        wos[nch][:],
                        wout_v[:, :, nch * 512:(nch + 1) * 512])

                ots = []
                for ti in range(NTT):
                    # V natural tiles: vnat[kk, kt, d] = V[kt*128+kk, d]
                    vnat = vnpool.tile([P, 16, P], bf16, tag="vnat")
                    for half in range(2):
                        tp = psQ.tile([P, 8 * P], bf16, tag="mm512")
                        for sb in range(8):
                            kt = half * 8 + sb
                            nc.tensor.transpose(
                                tp[:, sb * P:(sb + 1) * P],
                                vt_all[:, ti, kt * P:(kt + 1) * P],
                                ident[:])
                        nc.vector.tensor_copy(
                            vnat[:, half * 8:(half + 1) * 8, :].rearrange(
                                "p s d -> p (s d)").bitcast(f32),
                            tp[:].bitcast(f32))

                    ot = otpool.tile([P, 16, P], bf16, tag="ot")  # O^T
                    ots.append(ot)
                    for qc in range(4):
                        # Generations: (kt, width, s2 col, q offset) entries
                        # packed into one [128,<=1024] PSUM tile + one exp.
                        # Full k-tiles in pairs; causal k-tiles kt=4qc+r only
                        # cover live q cols [r*128, 512).
                        gens = []
                        for gp in range(2 * qc):
                            gens.append([(2 * gp, 512, 0, 0, False),
                                         (2 * gp + 1, 512, 512, 0, False)])
                        gens.append([(4 * qc, 512, 0, 0, True),
                                     (4 * qc + 1, 384, 512, 128, True)])
                        gens.append([(4 * qc + 2, 256, 0, 256, True),
                                     (4 * qc + 3, 128, 256, 384, True)])
                        nge = len(gens)

                        ot_ps = psO.tile([P, 512], f32, tag="otacc")
                        den_ps = psO.tile([P, 512], f32, tag="denacc")

                        pts = [None] * nge

                        def emit_qk(gi):
                            s2 = ps2.tile([P, 1024], f32, tag="s2")
                            totw = 0
                            for (kt, w, c0, q0, dg) in gens[gi]:
                                nc.tensor.matmul(
                                    s2[:, c0:c0 + w],
                                    kt_all[:, ti, kt * P:(kt + 1) * P],
                                    qt_all[:, ti,
                                           qc * 512 + q0:qc * 512 + q0 + w],
                                    start=True, stop=True)
                                totw = c0 + w
                            for (kt, w, c0, q0, dg) in gens[gi]:
                                if dg:
                                    nc.vector.tensor_tensor(
                                        s2[:, c0:c0 + P], s2[:, c0:c0 + P],
                                        tri[:], ALU.add)
                            pt = awpool.tile([P, 1024], bf16, tag="pt")
                            nc.scalar.activation(
                                pt[:, :totw], s2[:, :totw], AF.Exp,
                                bias=1.0, scale=SCALE)
                            pts[gi] = pt

                        def emit_pv(gi):
                            for ei, (kt, w, c0, q0, dg) in enumerate(gens[gi]):
                                first = gi == 0 and ei == 0
                                last = gi == nge - 1 and ei == 1
                                nc.tensor.matmul(
                                    ot_ps[:, q0:512],
                                    vnat[:, kt, :], pts[gi][:, c0:c0 + w],
                                    start=first, stop=last,
                                    skip_group_check=True)
                                nc.tensor.matmul(
                                    den_ps[:, q0:512],
                                    ones[:], pts[gi][:, c0:c0 + w],
                                    start=first, stop=last,
                                    skip_group_check=True)

                        emit_qk(0)
                        if nge > 1:
                            emit_qk(1)
                        for gi in range(nge):
                            emit_pv(gi)
                            if gi + 2 < nge:
                                emit_qk(gi + 2)
                        rec = rpool.tile([P, 512], f32, tag="rec")
                        nc.vector.reciprocal_approx_fast(
                            out=rec[:], in_=den_ps[:])
                        nc.vector.tensor_tensor(
                            ot[:, qc * 4:(qc + 1) * 4, :].rearrange(
                                "p s d -> p (s d)"),
                            ot_ps[:], rec[:], ALU.mult)

                # ---------------- output projection ----------------
                for nch in range(2, 4):
                    nc.sync.dma_start(
                        wos[nch][:],
                        wout_v[:, :, nch * 512:(nch + 1) * 512])
                for nch in range(4):
                    for ti in range(NTT):
                        lt = ots[ti].rearrange("d qt (i j) -> d qt i j", j=16)
                        ps = psQ.tile([P, 512], f32, tag="mm512")
                        for cc in range(16):
                            nc.tensor.matmul(
                                ps[:], lt[:, :, :, cc],
                                wos[nch][:, cc, :],
                                start=(cc == 0), stop=(cc == 15))
                        osb = ospool.tile([P, 512], f32, tag="osb")
                        nc.vector.tensor_copy(osb[:], ps[:])
                        nc.scalar.dma_start(
                            out.ap()[(g * NTT + ti) * P:
                                     (g * NTT + ti + 1) * P,
                                     nch * 512:(nch + 1) * 512], osb[:])
    nc.compile()
    return nc


def get_nc():
    if "nc" not in _NC_CACHE:
        _NC_CACHE["nc"] = build_nc()
    return _NC_CACHE["nc"]


def make_in_maps(x, w_qkv, w_out):
    import ml_dtypes

    bf = ml_dtypes.bfloat16
    xf = np.ascontiguousarray(np.asarray(x, dtype=np.float32)).reshape(
        B * S, E).astype(bf)
    wqkv_b = np.ascontiguousarray(
        np.asarray(w_qkv, dtype=np.float32).astype(bf))
    wout_b = np.ascontiguousarray(
        np.asarray(w_out, dtype=np.float32).astype(bf))
    # xt[c][kk, g, kc, ti, m] = x[c*1024 + g*512 + ti*128 + m, kc*128 + kk]
    xa = xf.reshape(NCORES, NGRP, NTT, P, 16, P).transpose(0, 5, 1, 4, 2, 3)
    in_maps = [
        {"xt": np.ascontiguousarray(xa[c]).reshape(P, NGRP * 16 * NTT * P),
         "wqkv": wqkv_b, "wout": wout_b}
        for c in range(NCORES)
    ]
    return in_maps


def kernel(x, w_qkv, w_out):
    from concourse.bass_utils import run_bass_kernel_spmd

    nc = get_nc()
    in_maps = make_in_maps(x, w_qkv, w_out)
    res = run_bass_kernel_spmd(nc, in_maps, core_ids=list(range(NCORES)))
    outs = [res.results[c]["out"] for c in range(NCORES)]
    return np.concatenate(outs, axis=0).reshape(B, S, E).astype(np.float32)


# revision 16
# speedup vs baseline: 1.0792x; 1.0053x over previous
"""Causal attention kernel for trn2, sharded over 8 NeuronCores.

Problem (B=4, S=2048, E=2048, H=16, D=128), fp32 in/out:
    qkv = x @ w_qkv; q,k,v = split(qkv)
    q,k,v reshaped (B,S,E)->(B,H,S,D) as a RAW view (no transpose), i.e.
    per (b,h): Q_h = rows [h*128,(h+1)*128) of q[b] reinterpreted [S,D].
    o = softmax(QK^T/sqrt(D) + causal(+1/-10000)) @ V, inverse raw view,
    out = o @ w_out.

The raw view maps head h to a contiguous block of 128 sequence rows, so
the computation splits into B*H = 64 independent tasks; core c gets 8
tasks = rows [c*1024,(c+1)*1024) of x.reshape(B*S, E).  No collectives.

All matmuls run in bf16 (inputs converted and x pre-transposed host-
side); accumulation is fp32 in PSUM.  Attention computes S^T = K Q^T per
(q-chunk 512, k-tile 128) with causal tiles cut to their live q-range,
exp batched per [128,<=1024] PSUM generation, and the softmax
denominator built by gpsimd adds over the exp tiles + one f32r matmul
(ones stationary) per q-chunk to broadcast the partition-dim sum.
"""

import numpy as np

B, S, E = 4, 2048, 2048
H, D, P = 16, 128, 128
NCORES = 8
ROWS = B * S // NCORES   # 1024 rows per core = 8 tasks of 128 rows
NGRP = 2                 # task groups per core
NTT = 4                  # tasks per group
SCALE = float(1.0 / np.sqrt(D))
NEG = -1.0e9  # pre-scale additive mask; exp underflows to exactly 0.0

_NC_CACHE = {}


def build_nc():
    import concourse.mybir as mybir
    import concourse.tile as tile
    from concourse import bacc
    from concourse.masks import make_identity

    f32 = mybir.dt.float32
    f32r = mybir.dt.float32r
    bf16 = mybir.dt.bfloat16
    AF = mybir.ActivationFunctionType
    ALU = mybir.AluOpType

    nc = bacc.Bacc("TRN2", target_bir_lowering=False, debug=False,
                   num_devices=NCORES)
    # xt[kk, g, kc, ti*128+m] = x[row g*512+ti*128+m, kc*128+kk] (host-
    # pretransposed, bf16): DMAs straight into the matmul-ready layout.
    xt = nc.dram_tensor("xt", [P, NGRP * 16 * NTT * P], bf16,
                        kind="ExternalInput")
    wqkv = nc.dram_tensor("wqkv", [E, 3 * E], bf16, kind="ExternalInput")
    wout = nc.dram_tensor("wout", [E, E], bf16, kind="ExternalInput")
    out = nc.dram_tensor("out", [ROWS, E], f32, kind="ExternalOutput")

    xt_v = xt.ap().rearrange("p (g kc tm) -> p g kc tm", g=NGRP, kc=16)
    wqkv_v = wqkv.ap().rearrange("(ko p) c -> p ko c", p=P)   # [128,16,6144]
    wout_v = wout.ap().rearrange("(co p) n -> p co n", p=P)   # [128,16,2048]

    with tile.TileContext(nc) as tc:
        with (
            tc.tile_pool(name="const", bufs=1) as cpool,
            tc.tile_pool(name="atp", bufs=2) as atpool,
            tc.tile_pool(name="qk", bufs=1) as qkpool,
            tc.tile_pool(name="ot", bufs=4) as otpool,
            tc.tile_pool(name="wq", bufs=2) as wqpool,
            tc.tile_pool(name="wo", bufs=4) as wopool,
            tc.tile_pool(name="attw", bufs=5) as awpool,
            tc.tile_pool(name="recp", bufs=2) as rpool,
            tc.tile_pool(name="vn", bufs=2) as vnpool,
            tc.tile_pool(name="osb", bufs=2) as ospool,
            tc.tile_pool(name="psQ", bufs=2, space="PSUM") as psQ,
            tc.tile_pool(name="ps2", bufs=2, space="PSUM") as ps2,
            tc.tile_pool(name="psO", bufs=1, space="PSUM") as psO,
        ):
            ident = cpool.tile([P, P], bf16, tag="ident")
            make_identity(nc, ident[:])
            # tri[kk, n] = 0 iff n >= kk else NEG: the within-tile causal
            # boundary (q-local offset n vs k-partition kk).
            tri = cpool.tile([P, P], f32, tag="tri")
            nc.gpsimd.memset(tri[:], 0.0)
            nc.gpsimd.affine_select(
                out=tri[:], in_=tri[:],
                compare_op=ALU.is_ge, fill=NEG,
                base=0, channel_multiplier=-1, pattern=[[1, P]],
            )
            # all-ones stationary (f32, matmul'd as f32r): den matmul
            # out[m,n] = sum_k ptsum[k,n] broadcast to all 128 partitions.
            ones = cpool.tile([P, P], bf16, tag="ones")
            nc.gpsimd.memset(ones[:], 1.0)

            at_g = [atpool.tile([P, 16, NTT * P], bf16, tag="at_all",
                                name=f"at{g}") for g in range(NGRP)]
            for kq in range(4):
                nc.sync.dma_start(at_g[0][:, kq * 4:(kq + 1) * 4, :],
                                  xt_v[:, 0, kq * 4:(kq + 1) * 4, :])
            wos = [wopool.tile([P, 16, 512], bf16, tag="wo",
                               name=f"wo{nch}") for nch in range(4)]
            for nch in range(4):
                nc.sync.dma_start(
                    wos[nch][:], wout_v[:, :, nch * 512:(nch + 1) * 512])

            # Output projection for one (task, nch) pair: a dense 16-matmul
            # chain, injected into later tasks' attention to fill exp-latency
            # bubbles on the PE.
            pending_oproj = []

            def emit_oproj_chain(row, ot_t, nch):
                lt = ot_t.rearrange("d qt (i j) -> d qt i j", j=16)
                ps = psQ.tile([P, 512], f32, tag="mm512")
                for cc in range(16):
                    nc.tensor.matmul(
                        ps[:], lt[:, :, :, cc], wos[nch][:, cc, :],
                        start=(cc == 0), stop=(cc == 15))
                osb = ospool.tile([P, 512], f32, tag="osb")
                nc.vector.tensor_copy(osb[:], ps[:])
                nc.scalar.dma_start(
                    out.ap()[row * P:(row + 1) * P,
                             nch * 512:(nch + 1) * 512], osb[:])

            def inject_oproj():
                if pending_oproj:
                    row, ot_t, nch = pending_oproj.pop(0)
                    emit_oproj_chain(row, ot_t, nch)

            for g in range(NGRP):
                qt_all = qkpool.tile([P, NTT, S], bf16, tag="qtc")
                kt_all = qkpool.tile([P, NTT, S], bf16, tag="ktc")
                vt_all = qkpool.tile([P, NTT, S], bf16, tag="vtc")
                dsts = (qt_all, kt_all, vt_all)

                # ---------------- QKV phase ----------------
                for cbp in range(24):
                    wq = wqpool.tile([P, 16, 2 * P], bf16, tag="wq")
                    nc.scalar.dma_start(
                        wq[:], wqkv_v[:, :, cbp * 2 * P:(cbp + 1) * 2 * P])
                    for half in range(2):
                        cb = cbp * 2 + half
                        ps = psQ.tile([P, NTT * P], f32, tag="mm512")
                        for kc in range(16):
                            nc.tensor.matmul(
                                ps[:],
                                wq[:, kc, half * P:(half + 1) * P],
                                at_g[g][:, kc, :],
                                start=(kc == 0), stop=(kc == 15))
                        j = cb % 16
                        nc.vector.tensor_copy(
                            dsts[cb // 16].rearrange(
                                "d t (i j) -> d t i j", j=16)[:, :, :, j],
                            ps[:].rearrange("d (t m) -> d t m", t=NTT))
                    if cbp % 6 == 5:
                        inject_oproj()

                # ---------------- attention phase (per task) ----------------
                if g + 1 < NGRP:
                    for kq in range(4):
                        nc.sync.dma_start(
                            at_g[g + 1][:, kq * 4:(kq + 1) * 4, :],
                            xt_v[:, g + 1, kq * 4:(kq + 1) * 4, :])
                ots = []
                for ti in range(NTT):
                    # V natural tiles: vnat[kk, kt, d] = V[kt*128+kk, d]
                    vnat = vnpool.tile([P, 16, P], bf16, tag="vnat")
                    for half in range(2):
                        tp = psQ.tile([P, 8 * P], bf16, tag="mm512")
                        for sb in range(8):
                            kt = half * 8 + sb
                            nc.tensor.transpose(
                                tp[:, sb * P:(sb + 1) * P],
                                vt_all[:, ti, kt * P:(kt + 1) * P],
                                ident[:])
                        nc.vector.tensor_copy(
                            vnat[:, half * 8:(half + 1) * 8, :].rearrange(
                                "p s d -> p (s d)").bitcast(f32),
                            tp[:].bitcast(f32))

                    ot = otpool.tile([P, 16, P], bf16, tag="ot")  # O^T
                    ots.append(ot)
                    for qc in range(4):
                        # Generations: (kt, width, s2 col, q offset) entries
                        # packed into one [128,<=1024] PSUM tile + one exp.
                        # Full k-tiles in pairs; causal k-tiles kt=4qc+r only
                        # cover live q cols [r*128, 512).
                        gens = []
                        for gp in range(2 * qc):
                            gens.append([(2 * gp, 512, 0, 0, False),
                                         (2 * gp + 1, 512, 512, 0, False)])
                        gens.append([(4 * qc, 512, 0, 0, True),
                                     (4 * qc + 1, 384, 512, 128, True)])
                        gens.append([(4 * qc + 2, 256, 0, 256, True),
                                     (4 * qc + 3, 128, 256, 384, True)])
                        nge = len(gens)

                        ot_ps = psO.tile([P, 512], f32, tag="otacc")
                        den_ps = psO.tile([P, 512], f32, tag="denacc")

                        pts = [None] * nge

                        def emit_qk(gi):
                            s2 = ps2.tile([P, 1024], f32, tag="s2")
                            totw = 0
                            for (kt, w, c0, q0, dg) in gens[gi]:
                                nc.tensor.matmul(
                                    s2[:, c0:c0 + w],
                                    kt_all[:, ti, kt * P:(kt + 1) * P],
                                    qt_all[:, ti,
                                           qc * 512 + q0:qc * 512 + q0 + w],
                                    start=True, stop=True)
                                totw = c0 + w
                            for (kt, w, c0, q0, dg) in gens[gi]:
                                if dg:
                                    nc.vector.tensor_tensor(
                                        s2[:, c0:c0 + P], s2[:, c0:c0 + P],
                                        tri[:], ALU.add)
                            pt = awpool.tile([P, 1024], bf16, tag="pt")
                            nc.scalar.activation(
                                pt[:, :totw], s2[:, :totw], AF.Exp,
                                bias=1.0, scale=SCALE)
                            pts[gi] = pt

                        def emit_pv(gi):
                            for ei, (kt, w, c0, q0, dg) in enumerate(gens[gi]):
                                first = gi == 0 and ei == 0
                                last = gi == nge - 1 and ei == 1
                                nc.tensor.matmul(
                                    ot_ps[:, q0:512],
                                    vnat[:, kt, :], pts[gi][:, c0:c0 + w],
                                    start=first, stop=last,
                                    skip_group_check=True)
                                nc.tensor.matmul(
                                    den_ps[:, q0:512],
                                    ones[:], pts[gi][:, c0:c0 + w],
                                    start=first, stop=last,
                                    skip_group_check=True)

                        emit_qk(0)
                        if nge > 1:
                            emit_qk(1)
                        inject_oproj()
                        for gi in range(nge):
                            emit_pv(gi)
                            if gi + 2 < nge:
                                emit_qk(gi + 2)
                        rec = rpool.tile([P, 512], f32, tag="rec")
                        nc.vector.reciprocal_approx_fast(
                            out=rec[:], in_=den_ps[:])
                        nc.vector.tensor_tensor(
                            ot[:, qc * 4:(qc + 1) * 4, :].rearrange(
                                "p s d -> p (s d)"),
                            ot_ps[:], rec[:], ALU.mult)

                    for nch in range(4):
                        pending_oproj.append((g * NTT + ti, ot, nch))

            while pending_oproj:
                inject_oproj()
    nc.compile()
    return nc


def get_nc():
    if "nc" not in _NC_CACHE:
        _NC_CACHE["nc"] = build_nc()
    return _NC_CACHE["nc"]


def make_in_maps(x, w_qkv, w_out):
    import ml_dtypes

    bf = ml_dtypes.bfloat16
    xf = np.ascontiguousarray(np.asarray(x, dtype=np.float32)).reshape(
        B * S, E).astype(bf)
    wqkv_b = np.ascontiguousarray(
        np.asarray(w_qkv, dtype=np.float32).astype(bf))
    wout_b = np.ascontiguousarray(
        np.asarray(w_out, dtype=np.float32).astype(bf))
    # xt[c][kk, g, kc, ti, m] = x[c*1024 + g*512 + ti*128 + m, kc*128 + kk]
    xa = xf.reshape(NCORES, NGRP, NTT, P, 16, P).transpose(0, 5, 1, 4, 2, 3)
    in_maps = [
        {"xt": np.ascontiguousarray(xa[c]).reshape(P, NGRP * 16 * NTT * P),
         "wqkv": wqkv_b, "wout": wout_b}
        for c in range(NCORES)
    ]
    return in_maps


def kernel(x, w_qkv, w_out):
    from concourse.bass_utils import run_bass_kernel_spmd

    nc = get_nc()
    in_maps = make_in_maps(x, w_qkv, w_out)
    res = run_bass_kernel_spmd(nc, in_maps, core_ids=list(range(NCORES)))
    outs = [res.results[c]["out"] for c in range(NCORES)]
    return np.concatenate(outs, axis=0).reshape(B, S, E).astype(np.float32)


# revision 17
# speedup vs baseline: 1.0949x; 1.0146x over previous
"""Causal attention kernel for trn2, sharded over 8 NeuronCores.

Problem (B=4, S=2048, E=2048, H=16, D=128), fp32 in/out:
    qkv = x @ w_qkv; q,k,v = split(qkv)
    q,k,v reshaped (B,S,E)->(B,H,S,D) as a RAW view (no transpose), i.e.
    per (b,h): Q_h = rows [h*128,(h+1)*128) of q[b] reinterpreted [S,D].
    o = softmax(QK^T/sqrt(D) + causal(+1/-10000)) @ V, inverse raw view,
    out = o @ w_out.

The raw view maps head h to a contiguous block of 128 sequence rows, so
the computation splits into B*H = 64 independent tasks; core c gets 8
tasks = rows [c*1024,(c+1)*1024) of x.reshape(B*S, E).  No collectives.

All matmuls run in bf16 (inputs converted and x pre-transposed host-
side); accumulation is fp32 in PSUM.  Attention computes S^T = K Q^T per
(q-chunk 512, k-tile 128) with causal tiles cut to their live q-range,
exp batched per [128,<=1024] PSUM generation, and the softmax
denominator built by gpsimd adds over the exp tiles + one f32r matmul
(ones stationary) per q-chunk to broadcast the partition-dim sum.
"""

import numpy as np

B, S, E = 4, 2048, 2048
H, D, P = 16, 128, 128
NCORES = 8
ROWS = B * S // NCORES   # 1024 rows per core = 8 tasks of 128 rows
NGRP = 2                 # task groups per core
NTT = 4                  # tasks per group
SCALE = float(1.0 / np.sqrt(D))
NEG = -1.0e9  # pre-scale additive mask; exp underflows to exactly 0.0

_NC_CACHE = {}


def build_nc():
    import concourse.mybir as mybir
    import concourse.tile as tile
    from concourse import bacc
    from concourse.masks import make_identity

    f32 = mybir.dt.float32
    f32r = mybir.dt.float32r
    bf16 = mybir.dt.bfloat16
    AF = mybir.ActivationFunctionType
    ALU = mybir.AluOpType

    nc = bacc.Bacc("TRN2", target_bir_lowering=False, debug=False,
                   num_devices=NCORES)
    # xt[kk, g, kc, ti*128+m] = x[row g*512+ti*128+m, kc*128+kk] (host-
    # pretransposed, bf16): DMAs straight into the matmul-ready layout.
    xt = nc.dram_tensor("xt", [P, NGRP * 16 * NTT * P], bf16,
                        kind="ExternalInput")
    wqkv = nc.dram_tensor("wqkv", [E, 3 * E], bf16, kind="ExternalInput")
    wout = nc.dram_tensor("wout", [E, E], bf16, kind="ExternalInput")
    out = nc.dram_tensor("out", [ROWS, E], f32, kind="ExternalOutput")

    xt_v = xt.ap().rearrange("p (g kc tm) -> p g kc tm", g=NGRP, kc=16)
    wqkv_v = wqkv.ap().rearrange("(ko p) c -> p ko c", p=P)   # [128,16,6144]
    wout_v = wout.ap().rearrange("(co p) n -> p co n", p=P)   # [128,16,2048]

    with tile.TileContext(nc) as tc:
        with (
            tc.tile_pool(name="const", bufs=1) as cpool,
            tc.tile_pool(name="atp", bufs=2) as atpool,
            tc.tile_pool(name="qk", bufs=1) as qkpool,
            tc.tile_pool(name="ot", bufs=4) as otpool,
            tc.tile_pool(name="wq", bufs=2) as wqpool,
            tc.tile_pool(name="wo", bufs=4) as wopool,
            tc.tile_pool(name="attw", bufs=5) as awpool,
            tc.tile_pool(name="recp", bufs=2) as rpool,
            tc.tile_pool(name="vn", bufs=2) as vnpool,
            tc.tile_pool(name="osb", bufs=2) as ospool,
            tc.tile_pool(name="psQ", bufs=2, space="PSUM") as psQ,
            tc.tile_pool(name="ps2", bufs=2, space="PSUM") as ps2,
            tc.tile_pool(name="psO", bufs=1, space="PSUM") as psO,
        ):
            ident = cpool.tile([P, P], bf16, tag="ident")
            make_identity(nc, ident[:])
            # tri[kk, n] = 0 iff n >= kk else NEG: the within-tile causal
            # boundary (q-local offset n vs k-partition kk).
            tri = cpool.tile([P, P], f32, tag="tri")
            nc.gpsimd.memset(tri[:], 0.0)
            nc.gpsimd.affine_select(
                out=tri[:], in_=tri[:],
                compare_op=ALU.is_ge, fill=NEG,
                base=0, channel_multiplier=-1, pattern=[[1, P]],
            )
            # all-ones stationary (f32, matmul'd as f32r): den matmul
            # out[m,n] = sum_k ptsum[k,n] broadcast to all 128 partitions.
            ones = cpool.tile([P, P], bf16, tag="ones")
            nc.gpsimd.memset(ones[:], 1.0)

            at_g = [atpool.tile([P, 16, NTT * P], bf16, tag="at_all",
                                name=f"at{g}") for g in range(NGRP)]
            for kq in range(4):
                nc.sync.dma_start(at_g[0][:, kq * 4:(kq + 1) * 4, :],
                                  xt_v[:, 0, kq * 4:(kq + 1) * 4, :])
            wos = [wopool.tile([P, 16, 512], bf16, tag="wo",
                               name=f"wo{nch}") for nch in range(4)]

            # Output projection for one (task, nch) pair: a dense 16-matmul
            # chain, injected into later tasks' attention to fill exp-latency
            # bubbles on the PE.
            pending_oproj = []

            def emit_oproj_chain(row, ot_t, nch):
                lt = ot_t.rearrange("d qt (i j) -> d qt i j", j=16)
                ps = psQ.tile([P, 512], f32, tag="mm512")
                for cc in range(16):
                    nc.tensor.matmul(
                        ps[:], lt[:, :, :, cc], wos[nch][:, cc, :],
                        start=(cc == 0), stop=(cc == 15))
                osb = ospool.tile([P, 512], f32, tag="osb")
                nc.vector.tensor_copy(osb[:], ps[:])
                nc.scalar.dma_start(
                    out.ap()[row * P:(row + 1) * P,
                             nch * 512:(nch + 1) * 512], osb[:])

            def inject_oproj():
                if pending_oproj:
                    row, ot_t, nch = pending_oproj.pop(0)
                    emit_oproj_chain(row, ot_t, nch)

            for g in range(NGRP):
                qt_all = qkpool.tile([P, NTT, S], bf16, tag="qtc")
                kt_all = qkpool.tile([P, NTT, S], bf16, tag="ktc")
                vt_all = qkpool.tile([P, NTT, S], bf16, tag="vtc")
                dsts = (qt_all, kt_all, vt_all)

                # ---------------- QKV phase ----------------
                for cbp in range(24):
                    wq = wqpool.tile([P, 16, 2 * P], bf16, tag="wq")
                    nc.scalar.dma_start(
                        wq[:], wqkv_v[:, :, cbp * 2 * P:(cbp + 1) * 2 * P])
                    for half in range(2):
                        cb = cbp * 2 + half
                        ps = psQ.tile([P, NTT * P], f32, tag="mm512")
                        for kc in range(16):
                            nc.tensor.matmul(
                                ps[:],
                                wq[:, kc, half * P:(half + 1) * P],
                                at_g[g][:, kc, :],
                                start=(kc == 0), stop=(kc == 15))
                        j = cb % 16
                        nc.vector.tensor_copy(
                            dsts[cb // 16].rearrange(
                                "d t (i j) -> d t i j", j=16)[:, :, :, j],
                            ps[:].rearrange("d (t m) -> d t m", t=NTT))
                    if cbp % 6 == 5:
                        inject_oproj()

                # ---------------- attention phase (per task) ----------------
                if g == 0:
                    for nch in range(4):
                        nc.sync.dma_start(
                            wos[nch][:],
                            wout_v[:, :, nch * 512:(nch + 1) * 512])
                if g + 1 < NGRP:
                    for kq in range(4):
                        nc.sync.dma_start(
                            at_g[g + 1][:, kq * 4:(kq + 1) * 4, :],
                            xt_v[:, g + 1, kq * 4:(kq + 1) * 4, :])
                ots = []
                for ti in range(NTT):
                    # V natural tiles: vnat[kk, kt, d] = V[kt*128+kk, d]
                    vnat = vnpool.tile([P, 16, P], bf16, tag="vnat")
                    for half in range(2):
                        tp = psQ.tile([P, 8 * P], bf16, tag="mm512")
                        for sb in range(8):
                            kt = half * 8 + sb
                            nc.tensor.transpose(
                                tp[:, sb * P:(sb + 1) * P],
                                vt_all[:, ti, kt * P:(kt + 1) * P],
                                ident[:])
                        nc.vector.tensor_copy(
                            vnat[:, half * 8:(half + 1) * 8, :].rearrange(
                                "p s d -> p (s d)").bitcast(f32),
                            tp[:].bitcast(f32))

                    ot = otpool.tile([P, 16, P], bf16, tag="ot")  # O^T
                    ots.append(ot)
                    for qc in range(4):
                        # Generations: (kt, width, s2 col, q offset) entries
                        # packed into one [128,<=1024] PSUM tile + one exp.
                        # Full k-tiles in pairs; causal k-tiles kt=4qc+r only
                        # cover live q cols [r*128, 512).
                        gens = []
                        for gp in range(2 * qc):
                            gens.append([(2 * gp, 512, 0, 0, False),
                                         (2 * gp + 1, 512, 512, 0, False)])
                        gens.append([(4 * qc, 512, 0, 0, True),
                                     (4 * qc + 1, 384, 512, 128, True)])
                        gens.append([(4 * qc + 2, 256, 0, 256, True),
                                     (4 * qc + 3, 128, 256, 384, True)])
                        nge = len(gens)

                        ot_ps = psO.tile([P, 512], f32, tag="otacc")
                        den_ps = psO.tile([P, 512], f32, tag="denacc")

                        pts = [None] * nge

                        def emit_qk(gi):
                            s2 = ps2.tile([P, 1024], f32, tag="s2")
                            totw = 0
                            for (kt, w, c0, q0, dg) in gens[gi]:
                                nc.tensor.matmul(
                                    s2[:, c0:c0 + w],
                                    kt_all[:, ti, kt * P:(kt + 1) * P],
                                    qt_all[:, ti,
                                           qc * 512 + q0:qc * 512 + q0 + w],
                                    start=True, stop=True)
                                totw = c0 + w
                            for (kt, w, c0, q0, dg) in gens[gi]:
                                if dg:
                                    nc.vector.tensor_tensor(
                                        s2[:, c0:c0 + P], s2[:, c0:c0 + P],
                                        tri[:], ALU.add)
                            pt = awpool.tile([P, 1024], bf16, tag="pt")
                            nc.scalar.activation(
                                pt[:, :totw], s2[:, :totw], AF.Exp,
                                bias=1.0, scale=SCALE)
                            pts[gi] = pt

                        def emit_pv(gi):
                            for ei, (kt, w, c0, q0, dg) in enumerate(gens[gi]):
                                first = gi == 0 and ei == 0
                                last = gi == nge - 1 and ei == 1
                                nc.tensor.matmul(
                                    ot_ps[:, q0:512],
                                    vnat[:, kt, :], pts[gi][:, c0:c0 + w],
                                    start=first, stop=last,
                                    skip_group_check=True)
                                nc.tensor.matmul(
                                    den_ps[:, q0:512],
                                    ones[:], pts[gi][:, c0:c0 + w],
                                    start=first, stop=last,
                                    skip_group_check=True)

                        emit_qk(0)
                        if nge > 1:
                            emit_qk(1)
                        inject_oproj()
                        for gi in range(nge):
                            emit_pv(gi)
                            if gi + 2 < nge:
                                emit_qk(gi + 2)
                        rec = rpool.tile([P, 512], f32, tag="rec")
                        nc.vector.reciprocal_approx_fast(
                            out=rec[:], in_=den_ps[:])
                        nc.vector.tensor_tensor(
                            ot[:, qc * 4:(qc + 1) * 4, :].rearrange(
                                "p s d -> p (s d)"),
                            ot_ps[:], rec[:], ALU.mult)

                    for nch in range(4):
                        pending_oproj.append((g * NTT + ti, ot, nch))

            while pending_oproj:
                inject_oproj()
    nc.compile()
    return nc


def get_nc():
    if "nc" not in _NC_CACHE:
        _NC_CACHE["nc"] = build_nc()
    return _NC_CACHE["nc"]


def make_in_maps(x, w_qkv, w_out):
    import ml_dtypes

    bf = ml_dtypes.bfloat16
    xf = np.ascontiguousarray(np.asarray(x, dtype=np.float32)).reshape(
        B * S, E).astype(bf)
    wqkv_b = np.ascontiguousarray(
        np.asarray(w_qkv, dtype=np.float32).astype(bf))
    wout_b = np.ascontiguousarray(
        np.asarray(w_out, dtype=np.float32).astype(bf))
    # xt[c][kk, g, kc, ti, m] = x[c*1024 + g*512 + ti*128 + m, kc*128 + kk]
    xa = xf.reshape(NCORES, NGRP, NTT, P, 16, P).transpose(0, 5, 1, 4, 2, 3)
    in_maps = [
        {"xt": np.ascontiguousarray(xa[c]).reshape(P, NGRP * 16 * NTT * P),
         "wqkv": wqkv_b, "wout": wout_b}
        for c in range(NCORES)
    ]
    return in_maps


def kernel(x, w_qkv, w_out):
    from concourse.bass_utils import run_bass_kernel_spmd

    nc = get_nc()
    in_maps = make_in_maps(x, w_qkv, w_out)
    res = run_bass_kernel_spmd(nc, in_maps, core_ids=list(range(NCORES)))
    outs = [res.results[c]["out"] for c in range(NCORES)]
    return np.concatenate(outs, axis=0).reshape(B, S, E).astype(np.float32)
